# revision 2
# baseline (speedup 1.0000x reference)
"""GNN SAGEConv (mean-agg) Trainium2 kernel v4, 8-core SPMD.

Per core (node-range dst-sharding, no collectives):
  - Host: edges split into 4 src-bank streams (int16 reach), dst-sorted
    within each; each stream starts with one all-dummy 128-edge chunk
    (yields guaranteed-zero csum rows).
  - Device mains: per 1024 edges one dma_gather (bf16 rows, 32B data in a
    256B-strided x table, one descriptor per edge, queues round-robin),
    one triangular matmul for the chunk-local (128-edge) prefix sums, and
    a strided store of the prefix rows to a 256B-strided csum table.
  - Boundary: agg[n] = sum_banks cs[D] - cs[E] + cs[P] where D/E are the
    stream positions of the last edge of node n / n-1 (forward-filled) and
    P the chunk-total fixup when D and E land in different chunks.  Read
    with node-major dma_gathers over <=32767-row csum sections.
  - Epilogue: out.T = W_l@mean.T + W_r@x.T (+b via the ones column), PE
    transposes + feature-major matmuls; counts ride feature 12.
"""

from contextlib import ExitStack

import numpy as np
import ml_dtypes

N_NODES = 100000
D = 12
DP = 16
NCORES = 8
NPAD = 13056
KN = NPAD // 128
BW = 25344          # src-bank width
BS = 25346          # x-table rows per bank (+zero row, pad)
NBANK = 4
GATHER_E = 1024     # edges per dma_gather
SEC_MAX = 31000     # max edges per boundary section

_MAX_WAITS = 1


def _apply_tile_patches(tile_mod, mybir, vector_clock):
    ScopedClock = vector_clock.ScopedClock

    def _drain_and_barrier(self, tick_clock, wait_clock):
        nc = self.nc
        probe = nc.sync.nop(hint="drain_wait_probe", nofuse=True)
        wait_clock.add_sem_waits(
            probe.ins, ScopedClock({None: tick_clock.global_clock})
        )
        si = probe.ins.sync_info
        waits = list(si.on_wait) if si is not None else []
        if len(waits) > _MAX_WAITS:
            si.on_wait = waits[:_MAX_WAITS]
            for i in range(_MAX_WAITS, len(waits), _MAX_WAITS):
                n = nc.sync.nop(hint="drain_wait_extra", nofuse=True)
                nsi = n.ins.sync_info
                if nsi is None:
                    n.ins.sync_info = mybir.SyncInfo(
                        on_wait=waits[i:i + _MAX_WAITS], on_update=[]
                    )
                else:
                    nsi.on_wait = waits[i:i + _MAX_WAITS]
        nc.sync.drain()
        nc.all_engine_barrier()
        assert self.sems is not None
        popped = nc._tile_sem_poison_stack.pop()
        assert popped is self._sem_poison
        nc.clear_and_free_semaphores(list(self.sems.allocated().values()))
        nc.all_engine_barrier()

    tile_mod.TileContext._drain_and_barrier = _drain_and_barrier


def _split_multi_waits(nc, mybir):
    cnt = 0
    for f in nc.m.functions:
        for bb in f.blocks:
            new = []
            for inst in bb.instructions:
                si = inst.sync_info
                waits = list(si.on_wait) if (si is not None and si.on_wait) else []
                if len(waits) > _MAX_WAITS:
                    extra, keep = waits[:-_MAX_WAITS], waits[-_MAX_WAITS:]
                    for j in range(0, len(extra), _MAX_WAITS):
                        nop = mybir.InstNoOp(name=f"waitsplit_{cnt}", ins=[], outs=[])
                        cnt += 1
                        nop.engine = inst.engine
                        nop.sync_info = mybir.SyncInfo(
                            on_wait=extra[j:j + _MAX_WAITS], on_update=[]
                        )
                        new.append(nop)
                    si.on_wait = keep
                new.append(inst)
            bb.instructions[:] = new


def _dma_gather_small(gp, bass, mybir, out_ap, in_ap, idxs_ap, num_idxs,
                      elem_size, elem_step, queue_num=0, num_reg=None):
    from concourse.ap_utils import ap_is_contiguous
    from concourse._compat import exact_div, round_up_to_multiple

    dtsz = mybir.dt.size(in_ap.dtype)
    assert in_ap.dtype == out_ap.dtype
    assert idxs_ap.dtype == mybir.dt.int16
    assert in_ap.space == bass.MemorySpace.DRAM
    assert ap_is_contiguous(in_ap.ap[1:])
    assert ap_is_contiguous(out_ap.ap[1:])
    assert ap_is_contiguous(idxs_ap.ap[1:])
    assert in_ap.ap[-1][1] == out_ap.ap[-1][1] == elem_size
    assert out_ap.ap[0][1] * out_ap.ap[1][1] == round_up_to_multiple(num_idxs, 128)
    assert in_ap.ap[0][0] == elem_step
    stride_bytes_256 = exact_div(elem_step * dtsz, 256)
    assert 0 < stride_bytes_256 < 256
    return gp.add_instruction(
        mybir.InstDMAGatherAnt(
            name=gp.bass.get_next_instruction_name(),
            ins=[*gp.lower_ap_dma(in_ap, for_custom_bir_dma=True),
                 gp.lower_ap(idxs_ap),
                 gp.lower_val_access(num_reg if num_reg is not None
                                     else gp.to_reg(num_idxs))],
            outs=[gp.lower_ap(out_ap)],
            transpose=False,
            num_idxs=num_idxs,
            elem_size=elem_size,
            stride_bytes_256=stride_bytes_256,
            gen_mode=0,
            single_packet=True,
            queue_num=queue_num,
            sbuf_tokens_per_rank=0,
            sbuf_free_dim_per_rank=0,
            sbuf_free_dim_pad_per_rank=0,
            sbuf_byte_offset=0,
        )
    )


def _pack_idx16(idx):
    n = len(idx)
    assert n % 16 == 0
    a = np.asarray(idx, np.int16).reshape(n // 16, 16).T
    return np.ascontiguousarray(np.tile(a, (8, 1)))


SECB = 8            # node blocks per boundary section (1024 nodes)
SECTIONS = [(k, min(k + SECB, KN)) for k in range(0, KN, SECB)]


def _core_section_edges(src_c, dst_c):
    """Per core: edge arrays per (bank, section), node-sorted."""
    bank = src_c // BW
    out = []
    for b in range(NBANK):
        m = bank == b
        s = (src_c[m] - b * BW).astype(np.int16)
        dl = dst_c[m]
        per = []
        for (kb0, kb1) in SECTIONS:
            msec = (dl >= kb0 * 128) & (dl < kb1 * 128)
            order = np.argsort(dl[msec], kind="stable")
            per.append((s[msec][order], dl[msec][order]))
        out.append(per)
    return out


def _fixed_layout(all_secs):
    """Static per-bank stream layout: per section a leading 128-row zero
    chunk + a slot sized to the max core demand (128-rounded)."""
    S = len(SECTIONS)
    Lsec = [[0] * S for _ in range(NBANK)]
    for per_core in all_secs:
        for b in range(NBANK):
            for si in range(S):
                Lsec[b][si] = max(Lsec[b][si], len(per_core[b][si][0]))
    plan = dict(ebase=[], secs=[[] for _ in range(NBANK)], slot=[], Lsec=Lsec)
    cur = 0
    for b in range(NBANK):
        plan["ebase"].append(cur)
        slots = []
        for si, (kb0, kb1) in enumerate(SECTIONS):
            L = -(-max(Lsec[b][si], 1) // 128) * 128
            Lsec[b][si] = L
            slot_start = cur + 128          # after the zero chunk
            assert L + 128 < 32600, (b, si, L)
            plan["secs"][b].append((kb0, kb1, slot_start - plan["ebase"][b] - 128))
            slots.append(slot_start)
            cur += 128 + L
        cur = -(-cur // GATHER_E) * GATHER_E
        plan["slot"].append(slots)
    plan["e_total"] = cur
    return plan


def _core_arrays(plan, per_core):
    """gidx stream + D/E/P node arrays (global rows) for one core."""
    gidx = np.full(plan["e_total"], BW, np.int16)
    dix = np.zeros((NBANK, NPAD), np.int64)
    eix = np.zeros((NBANK, NPAD), np.int64)
    pix = np.zeros((NBANK, NPAD), np.int64)
    for b in range(NBANK):
        for si, (kb0, kb1) in enumerate(SECTIONS):
            s, dl = per_core[b][si]
            st = plan["slot"][b][si]
            gidx[st:st + len(s)] = s
            cnt = np.bincount(dl - kb0 * 128, minlength=(kb1 - kb0) * 128)
            ends = st + np.cumsum(cnt)
            nzm = cnt > 0
            run = st - 1                    # zero-chunk last row
            nn_sec = (kb1 - kb0) * 128
            dd = np.empty(nn_sec, np.int64)
            for k in range(nn_sec):
                if nzm[k]:
                    run = ends[k] - 1
                dd[k] = run
            ee = np.empty(nn_sec, np.int64)
            ee[0] = st - 1
            ee[1:] = dd[:-1]
            cross = nzm & ((dd // 128) != (ee // 128))
            pp = np.where(cross, (ee // 128) * 128 + 127, st - 1)
            sl = slice(kb0 * 128, kb1 * 128)
            dix[b][sl], eix[b][sl], pix[b][sl] = dd, ee, pp
    return gidx, dix, eix, pix


def _host_check(plan, x, src_c, dst_c, nn):
    """numpy sim of the device pipeline (exact in f64)."""
    gidx = plan["gidx"].astype(np.int64)
    xt = np.zeros((NBANK * BS, DP))
    for b in range(NBANK):
        hi = min(BW, N_NODES - b * BW)
        xt[b * BS:b * BS + hi, :D] = x[b * BW:b * BW + hi]
        xt[b * BS:b * BS + hi, D] = 1.0
    # per-edge bank from position
    msgs = np.zeros((plan["e_total"], DP))
    for b in range(NBANK):
        e0 = plan["ebase"][b]
        e1 = plan["ebase"][b + 1] if b + 1 < NBANK else plan["e_total"]
        msgs[e0:e1] = xt[b * BS + gidx[e0:e1]]
    cs = msgs.reshape(-1, 128, DP).cumsum(axis=1).reshape(-1, DP)
    agg = np.zeros((NPAD, DP))
    for b in range(NBANK):
        e0 = plan["ebase"][b]
        agg += (cs[e0 + plan["dix"][b]] - cs[e0 + plan["eix"][b]]
                + cs[e0 + plan["pix"][b]])
    ref = np.zeros((NPAD, DP))
    np.add.at(ref[:, :D], dst_c, x[src_c])
    np.add.at(ref[:, D], dst_c, 1.0)
    return np.abs(agg - ref).max()


def _build_program(plan0):
    import concourse.bass as bass
    import concourse.mybir as mybir
    import concourse.tile as tile
    import concourse.vector_clock as vector_clock
    from concourse import library_config
    from concourse.library_overlay import lower_extended_insts

    _apply_tile_patches(tile, mybir, vector_clock)

    f32 = mybir.dt.float32
    bf16 = mybir.dt.bfloat16
    i16 = mybir.dt.int16

    E_TOT = plan0["e_total"]

    nc = bass.Bass(num_swdge_queues=4)
    xt = nc.declare_dram_parameter("xt", [NBANK * BS, 128], bf16, isOutput=False)
    gidx = nc.declare_dram_parameter("gidx", [128, E_TOT // 16], i16,
                                     isOutput=False)
    bidx = nc.declare_dram_parameter("bidx", [128, 12 * (NPAD // 16)], i16,
                                     isOutput=False)
    xs = nc.declare_dram_parameter("xs", [NPAD, DP], f32, isOutput=False)
    wl = nc.declare_dram_parameter("wl", [DP, D], f32, isOutput=False)
    wr = nc.declare_dram_parameter("wr", [DP, D], f32, isOutput=False)
    ltri = nc.declare_dram_parameter("ltri", [128, 128], bf16, isOutput=False)
    ident = nc.declare_dram_parameter("ident", [128, 128], f32, isOutput=False)
    out = nc.declare_dram_parameter("out", [12, NPAD], f32, isOutput=True)
    ebv = plan0["ebase"] + [E_TOT]
    csums = [nc.dram_tensor(f"csum{b}", [ebv[b + 1] - ebv[b], 128], bf16,
                            kind="Internal") for b in range(NBANK)]

    with ExitStack() as octx:
        tc = octx.enter_context(tile.TileContext(nc))
        const = octx.enter_context(tc.tile_pool(name="const", bufs=1))
        keep = octx.enter_context(tc.tile_pool(name="keep", bufs=1))

        nc.gpsimd.load_library(library_config.mlp)
        lt_t = const.tile([128, 128], bf16)
        nc.sync.dma_start(out=lt_t[:], in_=ltri[:])
        id_t = const.tile([128, 128], f32)
        nc.sync.dma_start(out=id_t[:], in_=ident[:])
        wl_t = const.tile([DP, D], f32)
        nc.sync.dma_start(out=wl_t[:], in_=wl[:])
        wr_t = const.tile([DP, D], f32)
        nc.sync.dma_start(out=wr_t[:], in_=wr[:])
        nreg = nc.gpsimd.to_reg(GATHER_E)
        bregs = {}

        dep = []
        bregs = {}
        with ExitStack() as pctx:
            gi_p = pctx.enter_context(tc.tile_pool(name="gi", bufs=6))
            msg_p = pctx.enter_context(tc.tile_pool(name="msg", bufs=6))
            cs_p = pctx.enter_context(tc.tile_pool(name="cs", bufs=6))
            ps_p = pctx.enter_context(
                tc.tile_pool(name="psph", bufs=6, space="PSUM"))
            gnum = 0
            for b in range(NBANK):
                e0b = plan0["ebase"][b]
                e1b = (plan0["ebase"][b + 1] if b + 1 < NBANK
                       else plan0["e_total"])
                Lb = e1b - e0b
                csv = csums[b].ap()[:, :DP].rearrange("(c p) f -> p c f", p=128)
                for e0 in range(e0b, e1b, GATHER_E):
                    ne = min(GATHER_E, e1b - e0)
                    nch = ne // 128
                    gi = gi_p.tile([128, GATHER_E // 16], i16, tag="gi")
                    nc.sync.dma_start(
                        out=gi[:, :ne // 16],
                        in_=gidx[:, e0 // 16:(e0 + ne) // 16])
                    msgs = msg_p.tile([128, GATHER_E // 128, DP], bf16,
                                      tag="m")
                    _dma_gather_small(
                        nc.gpsimd, bass, mybir, msgs[:, :nch, :],
                        xt[b * BS:(b + 1) * BS, :DP], gi[:, :ne // 16],
                        ne, DP, 128, queue_num=gnum % 4,
                        num_reg=nreg if ne == GATHER_E else None)
                    mm = ps_p.tile([128, (GATHER_E // 128) * DP], f32,
                                   tag="mm")
                    cw = nch * DP
                    nc.tensor.matmul(
                        mm[:, :cw], lt_t[:],
                        msgs[:, :nch, :].rearrange("p c f -> p (c f)"),
                        start=True, stop=True)
                    cst = cs_p.tile([128, (GATHER_E // 128) * DP], bf16,
                                    tag="cs")
                    if gnum % 2 == 0:
                        nc.vector.tensor_copy(cst[:, :cw], mm[:, :cw])
                    else:
                        nc.scalar.copy(cst[:, :cw], mm[:, :cw])
                    nc.sync.dma_start(
                        out=csv[:, (e0 - e0b) // 128:(e0 - e0b) // 128 + nch, :],
                        in_=cst[:, :cw].rearrange("p (c f) -> p c f", f=DP))
                    gnum += 1
                # boundary D/E/P for this bank overlap the next bank's mains
                for t in range(3):
                    k = b * 3 + t
                    ix = keep.tile([128, NPAD // 16], i16, tag=f"ix{k}")
                    nc.sync.dma_start(
                        out=ix[:],
                        in_=bidx[:, k * (NPAD // 16):(k + 1) * (NPAD // 16)])
                    tl = keep.tile([128, KN, DP], bf16, tag=f"dep{k}")
                    for (kb0, kb1, base) in plan0["secs"][b]:
                        nw = (kb1 - kb0) * 128
                        if nw not in bregs:
                            bregs[nw] = nc.gpsimd.to_reg(nw)
                        _dma_gather_small(
                            nc.gpsimd, bass, mybir, tl[:, kb0:kb1, :],
                            csums[b].ap()[base:min(base + 32600, Lb), :DP],
                            ix[:, kb0 * 8:kb1 * 8], nw, DP, 128,
                            queue_num=gnum % 4, num_reg=bregs[nw])
                        gnum += 1
                    dep.append(tl)

        agg = keep.tile([128, KN * DP], f32)
        tmp = keep.tile([128, KN * DP], f32)
        AOP = mybir.AluOpType
        for b in range(NBANK):
            dst_t = agg if b == 0 else tmp
            nc.vector.tensor_tensor(
                out=dst_t[:], in0=dep[3 * b][:].rearrange("p k f -> p (k f)"),
                in1=dep[3 * b + 1][:].rearrange("p k f -> p (k f)"),
                op=AOP.subtract)
            nc.vector.tensor_tensor(
                out=dst_t[:], in0=dst_t[:],
                in1=dep[3 * b + 2][:].rearrange("p k f -> p (k f)"),
                op=AOP.add)
            if b:
                nc.vector.tensor_add(out=agg[:], in0=agg[:], in1=tmp[:])

        # ---- epilogue ----
        rec = keep.tile([128, KN], f32)
        aggv = agg[:].rearrange("p (k f) -> p k f", f=DP)
        nc.vector.tensor_scalar_max(rec[:], aggv[:, :, D], 1.0)
        nc.vector.reciprocal(rec[:], rec[:])

        outT = keep.tile([12, NPAD], f32)
        xsv = xs[:].rearrange("(k p) f -> p k f", p=128)
        with ExitStack() as ectx:
            ep = ectx.enter_context(tc.tile_pool(name="ep", bufs=2))
            ps_sm = ectx.enter_context(
                tc.tile_pool(name="pse", bufs=1, space="PSUM"))
            groups = [(g * 4, min(4, KN - g * 4)) for g in range((KN + 3) // 4)]
            for g0, gw in groups:
                n_w = gw * 128
                xp = ep.tile([128, 4 * DP], f32, tag="xp")
                nc.sync.dma_start(
                    out=xp[:, :gw * DP].rearrange("p (k f) -> p k f", f=DP),
                    in_=xsv[:, g0:g0 + gw, :])
                mean = ep.tile([128, 4 * DP], f32, tag="mean")
                for t in range(gw):
                    nc.vector.tensor_scalar_mul(
                        mean[:, t * DP:(t + 1) * DP],
                        agg[:, (g0 + t) * DP:(g0 + t + 1) * DP],
                        rec[:, g0 + t:g0 + t + 1])
                aT_ps = ps_sm.tile([DP, 512], f32, tag="aT")
                xT_ps = ps_sm.tile([DP, 512], f32, tag="xT")
                for t in range(gw):
                    nc.tensor.transpose(
                        out=aT_ps[:, t * 128:(t + 1) * 128],
                        in_=mean[:, t * DP:(t + 1) * DP], identity=id_t[:])
                    nc.tensor.transpose(
                        out=xT_ps[:, t * 128:(t + 1) * 128],
                        in_=xp[:, t * DP:(t + 1) * DP], identity=id_t[:])
                aT = ep.tile([DP, 512], f32, tag="aTs")
                xT = ep.tile([DP, 512], f32, tag="xTs")
                nc.vector.tensor_copy(aT[:, :n_w], aT_ps[:, :n_w])
                nc.scalar.copy(xT[:, :n_w], xT_ps[:, :n_w])
                o1 = ps_sm.tile([12, 512], f32, tag="o1")
                nc.tensor.matmul(o1[:, :n_w], wl_t[:], aT[:, :n_w],
                                 start=True, stop=False)
                nc.tensor.matmul(o1[:, :n_w], wr_t[:], xT[:, :n_w],
                                 start=False, stop=True)
                nc.vector.tensor_copy(outT[:, g0 * 128:g0 * 128 + n_w],
                                      o1[:, :n_w])
        nc.sync.dma_start(out=out[:], in_=outT[:])

    _split_multi_waits(nc, mybir)
    lower_extended_insts(nc)
    return nc


def kernel(x, W_l, W_r, b, edge_index):
    from concourse.bass_utils import run_bass_kernel_spmd

    x = np.asarray(x, dtype=np.float32)
    W_l = np.asarray(W_l, dtype=np.float32)
    W_r = np.asarray(W_r, dtype=np.float32)
    b = np.asarray(b, dtype=np.float32)
    src = np.asarray(edge_index[0], dtype=np.int64)
    dst = np.asarray(edge_index[1], dtype=np.int64)
    E = src.shape[0]

    order = np.argsort(dst, kind="stable")
    src_s = src[order].astype(np.int64)
    dst_s = dst[order].astype(np.int64)

    pos = [0]
    for i in range(1, NCORES):
        t = (i * E) // NCORES
        v = dst_s[min(t, E - 1)]
        pos.append(int(np.searchsorted(dst_s, v, side="left")))
    pos.append(E)
    nb = [int(dst_s[pos[i]]) if pos[i] < E else N_NODES for i in range(NCORES)]
    nb.append(N_NODES)

    xt_np = np.zeros((NBANK * BS, 128), ml_dtypes.bfloat16)
    for bk in range(NBANK):
        hi = min(BW, N_NODES - bk * BW)
        xt_np[bk * BS:bk * BS + hi, :D] = x[bk * BW:bk * BW + hi]
        xt_np[bk * BS:bk * BS + hi, D] = 1.0

    wl_np = np.zeros((DP, D), np.float32)
    wl_np[:D, :] = W_l.T
    wr_np = np.zeros((DP, D), np.float32)
    wr_np[:D, :] = W_r.T
    wr_np[D, :] = b
    lt_np = np.triu(np.ones((128, 128))).astype(ml_dtypes.bfloat16)
    id_np = np.eye(128, dtype=np.float32)

    all_secs = []
    for i in range(NCORES):
        n0, n1 = nb[i], nb[i + 1]
        all_secs.append(_core_section_edges(
            src_s[pos[i]:pos[i + 1]], dst_s[pos[i]:pos[i + 1]] - n0))

    plan0 = _fixed_layout(all_secs)

    nc = _build_program(plan0)

    in_maps = []
    for i in range(NCORES):
        n0 = nb[i]
        gidx_full, dix, eix, pix = _core_arrays(plan0, all_secs[i])
        bidx = np.zeros((12, NPAD), np.int64)
        for bk in range(NBANK):
            eb = plan0["ebase"][bk]
            for (kb0, kb1, base) in plan0["secs"][bk]:
                sl = slice(kb0 * 128, kb1 * 128)
                for t, arr in ((0, dix), (1, eix), (2, pix)):
                    v = arr[bk][sl] - (eb + base)
                    assert (v >= 0).all() and (v < 32600).all(), (i, bk, kb0)
                    bidx[bk * 3 + t][sl] = v
        bidx16 = np.concatenate(
            [_pack_idx16(bidx[k]) for k in range(12)], axis=1)
        xs_np = np.zeros((NPAD, DP), np.float32)
        hi = min(NPAD, N_NODES - n0)
        xs_np[:hi, :D] = x[n0:n0 + hi]
        xs_np[:hi, D] = 1.0
        in_maps.append({
            "xt": xt_np, "gidx": _pack_idx16(gidx_full), "bidx": bidx16,
            "xs": xs_np, "wl": wl_np, "wr": wr_np, "ltri": lt_np,
            "ident": id_np,
        })

    try:
        res = run_bass_kernel_spmd(
            nc, in_maps, core_ids=list(range(NCORES)), trace=True)
    except ModuleNotFoundError:
        res = run_bass_kernel_spmd(
            nc, in_maps, core_ids=list(range(NCORES)), trace=False)
    if res.exec_time_ns:
        print(f"HW exec time: {res.exec_time_ns} ns")
    if res.instructions_and_trace:
        print("trace path:", res.instructions_and_trace[1])
    if res.profile_json:
        print("profile json:", res.profile_json)

    out = np.empty((N_NODES, D), dtype=np.float32)
    for i in range(NCORES):
        n0, n1 = nb[i], nb[i + 1]
        out[n0:n1, :] = res.results[i]["out"][:, :n1 - n0].T
    return out



# revision 5
# speedup vs baseline: 1.0982x; 1.0982x over previous
"""GNN SAGEConv (mean-agg) Trainium2 kernel v4, 8-core SPMD.

Per core (node-range dst-sharding, no collectives):
  - Host: edges split into 4 src-bank streams (int16 reach), dst-sorted
    within each; each stream starts with one all-dummy 128-edge chunk
    (yields guaranteed-zero csum rows).
  - Device mains: per 1024 edges one dma_gather (bf16 rows, 32B data in a
    256B-strided x table, one descriptor per edge, queues round-robin),
    one triangular matmul for the chunk-local (128-edge) prefix sums, and
    a strided store of the prefix rows to a 256B-strided csum table.
  - Boundary: agg[n] = sum_banks cs[D] - cs[E] + cs[P] where D/E are the
    stream positions of the last edge of node n / n-1 (forward-filled) and
    P the chunk-total fixup when D and E land in different chunks.  Read
    with node-major dma_gathers over <=32767-row csum sections.
  - Epilogue: out.T = W_l@mean.T + W_r@x.T (+b via the ones column), PE
    transposes + feature-major matmuls; counts ride feature 12.
"""

from contextlib import ExitStack

import numpy as np
import ml_dtypes

N_NODES = 100000
D = 12
DP = 16
NCORES = 8
NPAD = 13056
KN = NPAD // 128
BW = 25344          # src-bank width
BS = 25346          # x-table rows per bank (+zero row, pad)
NBANK = 4
GATHER_E = 1024     # edges per dma_gather
SEC_MAX = 31000     # max edges per boundary section

_MAX_WAITS = 1


def _apply_tile_patches(tile_mod, mybir, vector_clock):
    ScopedClock = vector_clock.ScopedClock

    def _drain_and_barrier(self, tick_clock, wait_clock):
        nc = self.nc
        probe = nc.sync.nop(hint="drain_wait_probe", nofuse=True)
        wait_clock.add_sem_waits(
            probe.ins, ScopedClock({None: tick_clock.global_clock})
        )
        si = probe.ins.sync_info
        waits = list(si.on_wait) if si is not None else []
        if len(waits) > _MAX_WAITS:
            si.on_wait = waits[:_MAX_WAITS]
            for i in range(_MAX_WAITS, len(waits), _MAX_WAITS):
                n = nc.sync.nop(hint="drain_wait_extra", nofuse=True)
                nsi = n.ins.sync_info
                if nsi is None:
                    n.ins.sync_info = mybir.SyncInfo(
                        on_wait=waits[i:i + _MAX_WAITS], on_update=[]
                    )
                else:
                    nsi.on_wait = waits[i:i + _MAX_WAITS]
        nc.sync.drain()
        nc.all_engine_barrier()
        assert self.sems is not None
        popped = nc._tile_sem_poison_stack.pop()
        assert popped is self._sem_poison
        nc.clear_and_free_semaphores(list(self.sems.allocated().values()))
        nc.all_engine_barrier()

    tile_mod.TileContext._drain_and_barrier = _drain_and_barrier


def _split_multi_waits(nc, mybir):
    cnt = 0
    for f in nc.m.functions:
        for bb in f.blocks:
            new = []
            for inst in bb.instructions:
                si = inst.sync_info
                waits = list(si.on_wait) if (si is not None and si.on_wait) else []
                if len(waits) > _MAX_WAITS:
                    extra, keep = waits[:-_MAX_WAITS], waits[-_MAX_WAITS:]
                    for j in range(0, len(extra), _MAX_WAITS):
                        nop = mybir.InstNoOp(name=f"waitsplit_{cnt}", ins=[], outs=[])
                        cnt += 1
                        nop.engine = inst.engine
                        nop.sync_info = mybir.SyncInfo(
                            on_wait=extra[j:j + _MAX_WAITS], on_update=[]
                        )
                        new.append(nop)
                    si.on_wait = keep
                new.append(inst)
            bb.instructions[:] = new


def _dma_gather_small(gp, bass, mybir, out_ap, in_ap, idxs_ap, num_idxs,
                      elem_size, elem_step, queue_num=0, num_reg=None):
    from concourse.ap_utils import ap_is_contiguous
    from concourse._compat import exact_div, round_up_to_multiple

    dtsz = mybir.dt.size(in_ap.dtype)
    assert in_ap.dtype == out_ap.dtype
    assert idxs_ap.dtype == mybir.dt.int16
    assert in_ap.space == bass.MemorySpace.DRAM
    assert ap_is_contiguous(in_ap.ap[1:])
    assert ap_is_contiguous(out_ap.ap[1:])
    assert ap_is_contiguous(idxs_ap.ap[1:])
    assert in_ap.ap[-1][1] == out_ap.ap[-1][1] == elem_size
    assert out_ap.ap[0][1] * out_ap.ap[1][1] == round_up_to_multiple(num_idxs, 128)
    assert in_ap.ap[0][0] == elem_step
    stride_bytes_256 = exact_div(elem_step * dtsz, 256)
    assert 0 < stride_bytes_256 < 256
    return gp.add_instruction(
        mybir.InstDMAGatherAnt(
            name=gp.bass.get_next_instruction_name(),
            ins=[*gp.lower_ap_dma(in_ap, for_custom_bir_dma=True),
                 gp.lower_ap(idxs_ap),
                 gp.lower_val_access(num_reg if num_reg is not None
                                     else gp.to_reg(num_idxs))],
            outs=[gp.lower_ap(out_ap)],
            transpose=False,
            num_idxs=num_idxs,
            elem_size=elem_size,
            stride_bytes_256=stride_bytes_256,
            gen_mode=0,
            single_packet=True,
            queue_num=queue_num,
            sbuf_tokens_per_rank=0,
            sbuf_free_dim_per_rank=0,
            sbuf_free_dim_pad_per_rank=0,
            sbuf_byte_offset=0,
        )
    )


def _pack_idx16(idx):
    n = len(idx)
    assert n % 16 == 0
    a = np.asarray(idx, np.int16).reshape(n // 16, 16).T
    return np.ascontiguousarray(np.tile(a, (8, 1)))


SECB = 8            # node blocks per boundary section (1024 nodes)
SECTIONS = [(k, min(k + SECB, KN)) for k in range(0, KN, SECB)]


def _core_section_edges(src_c, dst_c):
    """Per core: edge arrays per (bank, section), node-sorted."""
    bank = src_c // BW
    out = []
    for b in range(NBANK):
        m = bank == b
        s = (src_c[m] - b * BW).astype(np.int16)
        dl = dst_c[m]
        per = []
        for (kb0, kb1) in SECTIONS:
            msec = (dl >= kb0 * 128) & (dl < kb1 * 128)
            order = np.argsort(dl[msec], kind="stable")
            per.append((s[msec][order], dl[msec][order]))
        out.append(per)
    return out


def _fixed_layout(all_secs):
    """Static per-bank stream layout: per section a leading 128-row zero
    chunk + a slot sized to the max core demand (128-rounded)."""
    S = len(SECTIONS)
    Lsec = [[0] * S for _ in range(NBANK)]
    for per_core in all_secs:
        for b in range(NBANK):
            for si in range(S):
                Lsec[b][si] = max(Lsec[b][si], len(per_core[b][si][0]))
    plan = dict(ebase=[], secs=[[] for _ in range(NBANK)], slot=[], Lsec=Lsec)
    cur = 0
    for b in range(NBANK):
        plan["ebase"].append(cur)
        slots = []
        for si, (kb0, kb1) in enumerate(SECTIONS):
            L = -(-max(Lsec[b][si], 1) // 128) * 128
            Lsec[b][si] = L
            slot_start = cur + 128          # after the zero chunk
            assert L + 128 < 32600, (b, si, L)
            plan["secs"][b].append((kb0, kb1, slot_start - plan["ebase"][b] - 128))
            slots.append(slot_start)
            cur += 128 + L
        cur = -(-cur // GATHER_E) * GATHER_E
        plan["slot"].append(slots)
    plan["e_total"] = cur
    return plan


def _core_arrays(plan, per_core):
    """gidx stream + D/E/P node arrays (global rows) for one core."""
    gidx = np.full(plan["e_total"], BW, np.int16)
    dix = np.zeros((NBANK, NPAD), np.int64)
    eix = np.zeros((NBANK, NPAD), np.int64)
    pix = np.zeros((NBANK, NPAD), np.int64)
    for b in range(NBANK):
        for si, (kb0, kb1) in enumerate(SECTIONS):
            s, dl = per_core[b][si]
            st = plan["slot"][b][si]
            gidx[st:st + len(s)] = s
            cnt = np.bincount(dl - kb0 * 128, minlength=(kb1 - kb0) * 128)
            ends = st + np.cumsum(cnt)
            nzm = cnt > 0
            run = st - 1                    # zero-chunk last row
            nn_sec = (kb1 - kb0) * 128
            dd = np.empty(nn_sec, np.int64)
            for k in range(nn_sec):
                if nzm[k]:
                    run = ends[k] - 1
                dd[k] = run
            ee = np.empty(nn_sec, np.int64)
            ee[0] = st - 1
            ee[1:] = dd[:-1]
            cross = nzm & ((dd // 128) != (ee // 128))
            pp = np.where(cross, (ee // 128) * 128 + 127, st - 1)
            sl = slice(kb0 * 128, kb1 * 128)
            dix[b][sl], eix[b][sl], pix[b][sl] = dd, ee, pp
    return gidx, dix, eix, pix


def _host_check(plan, x, src_c, dst_c, nn):
    """numpy sim of the device pipeline (exact in f64)."""
    gidx = plan["gidx"].astype(np.int64)
    xt = np.zeros((NBANK * BS, DP))
    for b in range(NBANK):
        hi = min(BW, N_NODES - b * BW)
        xt[b * BS:b * BS + hi, :D] = x[b * BW:b * BW + hi]
        xt[b * BS:b * BS + hi, D] = 1.0
    # per-edge bank from position
    msgs = np.zeros((plan["e_total"], DP))
    for b in range(NBANK):
        e0 = plan["ebase"][b]
        e1 = plan["ebase"][b + 1] if b + 1 < NBANK else plan["e_total"]
        msgs[e0:e1] = xt[b * BS + gidx[e0:e1]]
    cs = msgs.reshape(-1, 128, DP).cumsum(axis=1).reshape(-1, DP)
    agg = np.zeros((NPAD, DP))
    for b in range(NBANK):
        e0 = plan["ebase"][b]
        agg += (cs[e0 + plan["dix"][b]] - cs[e0 + plan["eix"][b]]
                + cs[e0 + plan["pix"][b]])
    ref = np.zeros((NPAD, DP))
    np.add.at(ref[:, :D], dst_c, x[src_c])
    np.add.at(ref[:, D], dst_c, 1.0)
    return np.abs(agg - ref).max()


def _build_program(plan0):
    import concourse.bass as bass
    import concourse.mybir as mybir
    import concourse.tile as tile
    import concourse.vector_clock as vector_clock
    from concourse import library_config
    from concourse.library_overlay import lower_extended_insts

    _apply_tile_patches(tile, mybir, vector_clock)

    f32 = mybir.dt.float32
    bf16 = mybir.dt.bfloat16
    i16 = mybir.dt.int16

    E_TOT = plan0["e_total"]

    nc = bass.Bass(num_swdge_queues=4)
    xt = nc.declare_dram_parameter("xt", [NBANK * BS, 128], bf16, isOutput=False)
    gidx = nc.declare_dram_parameter("gidx", [128, E_TOT // 16], i16,
                                     isOutput=False)
    bidx = nc.declare_dram_parameter("bidx", [128, 12 * (NPAD // 16)], i16,
                                     isOutput=False)
    xs = nc.declare_dram_parameter("xs", [NPAD, DP], f32, isOutput=False)
    wl = nc.declare_dram_parameter("wl", [DP, D], f32, isOutput=False)
    wr = nc.declare_dram_parameter("wr", [DP, D], f32, isOutput=False)
    ltri = nc.declare_dram_parameter("ltri", [128, 128], bf16, isOutput=False)
    ident = nc.declare_dram_parameter("ident", [128, 128], f32, isOutput=False)
    out = nc.declare_dram_parameter("out", [12, NPAD], f32, isOutput=True)
    ebv = plan0["ebase"] + [E_TOT]
    csums = [nc.dram_tensor(f"csum{b}", [ebv[b + 1] - ebv[b], 128], bf16,
                            kind="Internal") for b in range(NBANK)]

    with ExitStack() as octx:
        tc = octx.enter_context(tile.TileContext(nc))
        const = octx.enter_context(tc.tile_pool(name="const", bufs=1))
        keep = octx.enter_context(tc.tile_pool(name="keep", bufs=1))

        nc.gpsimd.load_library(library_config.mlp)
        lt_t = const.tile([128, 128], bf16)
        nc.sync.dma_start(out=lt_t[:], in_=ltri[:])
        id_t = const.tile([128, 128], f32)
        nc.sync.dma_start(out=id_t[:], in_=ident[:])
        wl_t = const.tile([DP, D], f32)
        nc.sync.dma_start(out=wl_t[:], in_=wl[:])
        wr_t = const.tile([DP, D], f32)
        nc.sync.dma_start(out=wr_t[:], in_=wr[:])
        nreg = nc.gpsimd.to_reg(GATHER_E)
        bregs = {}

        dep = []
        bregs = {}
        with ExitStack() as pctx:
            gi_p = pctx.enter_context(tc.tile_pool(name="gi", bufs=6))
            msg_p = pctx.enter_context(tc.tile_pool(name="msg", bufs=6))
            cs_p = pctx.enter_context(tc.tile_pool(name="cs", bufs=6))
            ps_p = pctx.enter_context(
                tc.tile_pool(name="psph", bufs=6, space="PSUM"))
            gnum = 0
            for b in range(NBANK):
                e0b = plan0["ebase"][b]
                e1b = (plan0["ebase"][b + 1] if b + 1 < NBANK
                       else plan0["e_total"])
                Lb = e1b - e0b
                csv = csums[b].ap()[:, :DP].rearrange("(c p) f -> p c f", p=128)
                for e0 in range(e0b, e1b, GATHER_E):
                    ne = min(GATHER_E, e1b - e0)
                    nch = ne // 128
                    gi = gi_p.tile([128, GATHER_E // 16], i16, tag="gi")
                    nc.sync.dma_start(
                        out=gi[:, :ne // 16],
                        in_=gidx[:, e0 // 16:(e0 + ne) // 16])
                    msgs = msg_p.tile([128, GATHER_E // 128, DP], bf16,
                                      tag="m")
                    _dma_gather_small(
                        nc.gpsimd, bass, mybir, msgs[:, :nch, :],
                        xt[b * BS:(b + 1) * BS, :DP], gi[:, :ne // 16],
                        ne, DP, 128, queue_num=gnum % 4,
                        num_reg=nreg if ne == GATHER_E else None)
                    mm = ps_p.tile([128, (GATHER_E // 128) * DP], f32,
                                   tag="mm")
                    cw = nch * DP
                    nc.tensor.matmul(
                        mm[:, :cw], lt_t[:],
                        msgs[:, :nch, :].rearrange("p c f -> p (c f)"),
                        start=True, stop=True)
                    cst = cs_p.tile([128, (GATHER_E // 128) * DP], bf16,
                                    tag="cs")
                    if gnum % 2 == 0:
                        nc.vector.tensor_copy(cst[:, :cw], mm[:, :cw])
                    else:
                        nc.scalar.copy(cst[:, :cw], mm[:, :cw])
                    nc.sync.dma_start(
                        out=csv[:, (e0 - e0b) // 128:(e0 - e0b) // 128 + nch, :],
                        in_=cst[:, :cw].rearrange("p (c f) -> p c f", f=DP))
                    gnum += 1
                # boundary D/E/P for this bank overlap the next bank's mains
                for t in range(3):
                    k = b * 3 + t
                    ix = keep.tile([128, NPAD // 16], i16, tag=f"ix{k}")
                    nc.sync.dma_start(
                        out=ix[:],
                        in_=bidx[:, k * (NPAD // 16):(k + 1) * (NPAD // 16)])
                    tl = keep.tile([128, KN, DP], bf16, tag=f"dep{k}")
                    for (kb0, kb1, base) in plan0["secs"][b]:
                        nw = (kb1 - kb0) * 128
                        if nw not in bregs:
                            bregs[nw] = nc.gpsimd.to_reg(nw)
                        _dma_gather_small(
                            nc.gpsimd, bass, mybir, tl[:, kb0:kb1, :],
                            csums[b].ap()[base:min(base + 32600, Lb), :DP],
                            ix[:, kb0 * 8:kb1 * 8], nw, DP, 128,
                            queue_num=gnum % 4, num_reg=bregs[nw])
                        gnum += 1
                    dep.append(tl)

        agg = keep.tile([128, KN * DP], f32)
        tmp = keep.tile([128, KN * DP], f32)
        AOP = mybir.AluOpType
        for b in range(NBANK):
            dst_t = agg if b == 0 else tmp
            nc.vector.tensor_tensor(
                out=dst_t[:], in0=dep[3 * b][:].rearrange("p k f -> p (k f)"),
                in1=dep[3 * b + 1][:].rearrange("p k f -> p (k f)"),
                op=AOP.subtract)
            nc.vector.tensor_tensor(
                out=dst_t[:], in0=dst_t[:],
                in1=dep[3 * b + 2][:].rearrange("p k f -> p (k f)"),
                op=AOP.add)
            if b:
                nc.vector.tensor_add(out=agg[:], in0=agg[:], in1=tmp[:])

        # ---- epilogue ----
        rec = keep.tile([128, KN], f32)
        aggv = agg[:].rearrange("p (k f) -> p k f", f=DP)
        nc.vector.tensor_scalar_max(rec[:], aggv[:, :, D], 1.0)
        nc.vector.reciprocal(rec[:], rec[:])

        outT = keep.tile([12, NPAD], f32)
        xsv = xs[:].rearrange("(k p) f -> p k f", p=128)
        with ExitStack() as ectx:
            ep = ectx.enter_context(tc.tile_pool(name="ep", bufs=2))
            ps_sm = ectx.enter_context(
                tc.tile_pool(name="pse", bufs=1, space="PSUM"))
            groups = [(g * 4, min(4, KN - g * 4)) for g in range((KN + 3) // 4)]
            for g0, gw in groups:
                n_w = gw * 128
                xp = ep.tile([128, 4 * DP], f32, tag="xp")
                nc.sync.dma_start(
                    out=xp[:, :gw * DP].rearrange("p (k f) -> p k f", f=DP),
                    in_=xsv[:, g0:g0 + gw, :])
                mean = ep.tile([128, 4 * DP], f32, tag="mean")
                for t in range(gw):
                    nc.vector.tensor_scalar_mul(
                        mean[:, t * DP:(t + 1) * DP],
                        agg[:, (g0 + t) * DP:(g0 + t + 1) * DP],
                        rec[:, g0 + t:g0 + t + 1])
                aT_ps = ps_sm.tile([DP, 512], f32, tag="aT")
                xT_ps = ps_sm.tile([DP, 512], f32, tag="xT")
                for t in range(gw):
                    nc.tensor.transpose(
                        out=aT_ps[:, t * 128:(t + 1) * 128],
                        in_=mean[:, t * DP:(t + 1) * DP], identity=id_t[:])
                    nc.tensor.transpose(
                        out=xT_ps[:, t * 128:(t + 1) * 128],
                        in_=xp[:, t * DP:(t + 1) * DP], identity=id_t[:])
                aT = ep.tile([DP, 512], f32, tag="aTs")
                xT = ep.tile([DP, 512], f32, tag="xTs")
                nc.vector.tensor_copy(aT[:, :n_w], aT_ps[:, :n_w])
                nc.scalar.copy(xT[:, :n_w], xT_ps[:, :n_w])
                o1 = ps_sm.tile([12, 512], f32, tag="o1")
                nc.tensor.matmul(o1[:, :n_w], wl_t[:], aT[:, :n_w],
                                 start=True, stop=False)
                nc.tensor.matmul(o1[:, :n_w], wr_t[:], xT[:, :n_w],
                                 start=False, stop=True)
                nc.vector.tensor_copy(outT[:, g0 * 128:g0 * 128 + n_w],
                                      o1[:, :n_w])
        nc.sync.dma_start(out=out[:], in_=outT[:])

    _split_multi_waits(nc, mybir)
    lower_extended_insts(nc)
    return nc


def kernel(x, W_l, W_r, b, edge_index):
    from concourse.bass_utils import run_bass_kernel_spmd

    x = np.asarray(x, dtype=np.float32)
    W_l = np.asarray(W_l, dtype=np.float32)
    W_r = np.asarray(W_r, dtype=np.float32)
    b = np.asarray(b, dtype=np.float32)
    src = np.asarray(edge_index[0], dtype=np.int64)
    dst = np.asarray(edge_index[1], dtype=np.int64)
    E = src.shape[0]

    order = np.argsort(dst, kind="stable")
    src_s = src[order].astype(np.int64)
    dst_s = dst[order].astype(np.int64)

    pos = [0]
    for i in range(1, NCORES):
        t = (i * E) // NCORES
        v = dst_s[min(t, E - 1)]
        pos.append(int(np.searchsorted(dst_s, v, side="left")))
    pos.append(E)
    nb = [int(dst_s[pos[i]]) if pos[i] < E else N_NODES for i in range(NCORES)]
    nb.append(N_NODES)

    xt_np = np.zeros((NBANK * BS, 128), ml_dtypes.bfloat16)
    for bk in range(NBANK):
        hi = min(BW, N_NODES - bk * BW)
        xt_np[bk * BS:bk * BS + hi, :D] = x[bk * BW:bk * BW + hi]
        xt_np[bk * BS:bk * BS + hi, D] = 1.0

    wl_np = np.zeros((DP, D), np.float32)
    wl_np[:D, :] = W_l.T
    wr_np = np.zeros((DP, D), np.float32)
    wr_np[:D, :] = W_r.T
    wr_np[D, :] = b
    lt_np = np.triu(np.ones((128, 128))).astype(ml_dtypes.bfloat16)
    id_np = np.eye(128, dtype=np.float32)

    all_secs = []
    for i in range(NCORES):
        n0, n1 = nb[i], nb[i + 1]
        all_secs.append(_core_section_edges(
            src_s[pos[i]:pos[i + 1]], dst_s[pos[i]:pos[i + 1]] - n0))

    plan0 = _fixed_layout(all_secs)

    nc = _build_program(plan0)

    in_maps = []
    for i in range(NCORES):
        n0 = nb[i]
        gidx_full, dix, eix, pix = _core_arrays(plan0, all_secs[i])
        bidx = np.zeros((12, NPAD), np.int64)
        for bk in range(NBANK):
            eb = plan0["ebase"][bk]
            for (kb0, kb1, base) in plan0["secs"][bk]:
                sl = slice(kb0 * 128, kb1 * 128)
                for t, arr in ((0, dix), (1, eix), (2, pix)):
                    v = arr[bk][sl] - (eb + base)
                    assert (v >= 0).all() and (v < 32600).all(), (i, bk, kb0)
                    bidx[bk * 3 + t][sl] = v
        bidx16 = np.concatenate(
            [_pack_idx16(bidx[k]) for k in range(12)], axis=1)
        xs_np = np.zeros((NPAD, DP), np.float32)
        hi = min(NPAD, N_NODES - n0)
        xs_np[:hi, :D] = x[n0:n0 + hi]
        xs_np[:hi, D] = 1.0
        in_maps.append({
            "xt": xt_np, "gidx": _pack_idx16(gidx_full), "bidx": bidx16,
            "xs": xs_np, "wl": wl_np, "wr": wr_np, "ltri": lt_np,
            "ident": id_np,
        })

    try:
        res = run_bass_kernel_spmd(
            nc, in_maps, core_ids=list(range(NCORES)), trace=True)
    except ModuleNotFoundError:
        res = run_bass_kernel_spmd(
            nc, in_maps, core_ids=list(range(NCORES)), trace=False)
    if res.exec_time_ns:
        print(f"HW exec time: {res.exec_time_ns} ns")
    if res.instructions_and_trace:
        print("trace path:", res.instructions_and_trace[1])
    if res.profile_json:
        print("profile json:", res.profile_json)

    out = np.empty((N_NODES, D), dtype=np.float32)
    for i in range(NCORES):
        n0, n1 = nb[i], nb[i + 1]
        out[n0:n1, :] = res.results[i]["out"][:, :n1 - n0].T
    return out

